# revision 19
# baseline (speedup 1.0000x reference)
"""Trainium2 Bass kernel for an 8-expert top-2 MoE layer (expert-parallel).

Problem (hardcoded): x:(4,2048,1024) f32, gate_w:(1024,8), w1:(8,1024,4096),
b1:(8,4096), w2:(8,4096,1024), b2:(8,1024).  Returns (out, aux_loss) like the
reference:
    logits = x @ gate_w ; probs = softmax ; top-2 renormalized
    out    = sum_e coef_e * (gelu(x @ w1[e] + b1[e]) @ w2[e] + b2[e])
    aux    = E * sum(f * P)

Strategy:
  * Router / softmax / top-2 / aux-loss on host (0.1% of the FLOPs).
  * Expert-parallel: core e runs expert e's FFN over the tokens routed to it
    (gathered + padded to capacity C on host, "all-to-all dispatch" done
    host-side since the kernel receives full inputs anyway).
  * Per core: yT = w2.T @ gelu(w1.T @ xT + b1) + b2 with all matmuls in
    float32r (full-rate fp32 path of the PE, ~1e-4 rel err).
    Layouts are chosen so no transposes are ever needed on device:
    stationary = weight tiles as stored, moving = token panels.
  * H is processed in 4 chunks of 1024 so the h1 activations for one chunk
    (all token blocks) stay SBUF-resident; the second matmul accumulates
    chunk partials straight into DRAM via DMA accum_op=add.
  * Host combines: out[tok] += coef * yT.T, weighted by routing weights.
"""
import os
import sys

sys.path.insert(0, "/opt/trn_rl_repo")

from contextlib import ExitStack

import numpy as np

import concourse.bacc as bacc
import concourse.bass as bass
import concourse.tile as tile
from concourse import mybir

B, L, D = 4, 2048, 1024
E, H, K = 8, 4096, 2
P = 128
NCORES = 8
TB = 512            # token block (matmul moving free dim; one PSUM bank)
HC = 1024           # H chunk held in SBUF (8 k-tiles)
DK = D // P         # 8 k-tiles over D
HM = H // P         # 32 m-tiles over H
DM = D // P         # 8 m-tiles over D
NCH = H // HC       # 4 H-chunks

TRACE = bool(int(os.environ.get("BASSK_TRACE", "0")))
LAST_RESULTS = None

_prog_cache = {}


def _install_prof_shim():
    """Enable NTFF profiling under axon (the agent image's antenv lacks the
    hook) and keep profile artifacts local."""
    import types
    import concourse.bass_utils as bu

    if "antenv.axon_hooks" not in sys.modules:
        holder = {"hook": None}
        mod = types.ModuleType("antenv.axon_hooks")
        mod.set_axon_ntff_profile_hook = lambda h: holder.update(hook=h)
        mod.get_axon_ntff_profile_hook = lambda: holder["hook"]
        sys.modules["antenv.axon_hooks"] = mod
        from trn_agent_boot.trn_boot import _ntff_profile_via_ctypes

        mod.set_axon_ntff_profile_hook(
            _ntff_profile_via_ctypes("/opt/axon/libaxon_pjrt.so")
        )
        bu.upload_artifacts = lambda tmpdir: f"local://{tmpdir}"


def build_program(C):
    """One SPMD program: yT = w2.T @ gelu(w1.T @ xT + b1) + b2, xT:[D,C]."""
    # Decompose C into token blocks of 512/384/256 — fp32r matmuls need a
    # moving free dim >=256 to run at full rate (1 cycle/row).
    assert C % 128 == 0 and C >= 256
    sizes = [TB] * (C // TB)
    r = C % TB
    if r == 128:
        sizes = sizes[:-1] + [384, 256]
    elif r:
        sizes.append(r)
    blocks = []
    off = 0
    for sz in sizes:
        blocks.append((off, sz))
        off += sz
    assert off == C
    nt = len(blocks)
    f32r = mybir.dt.float32r
    f32 = mybir.dt.float32
    AF = mybir.ActivationFunctionType

    nc = bacc.Bacc("TRN2", target_bir_lowering=False, debug=False,
                   num_devices=NCORES)
    xT = nc.dram_tensor("xT", [D, C], f32r, kind="ExternalInput").ap()
    w1 = nc.dram_tensor("w1", [DK, P, H], f32r, kind="ExternalInput").ap()
    w2 = nc.dram_tensor("w2", [HM, P, D], f32r, kind="ExternalInput").ap()
    b1 = nc.dram_tensor("b1", [H], f32, kind="ExternalInput").ap()
    b2 = nc.dram_tensor("b2", [D], f32, kind="ExternalInput").ap()
    yT = nc.dram_tensor("yT", [D, C], f32, kind="ExternalOutput").ap()

    nsz = {}
    for _, sz in blocks:
        nsz[sz] = nsz.get(sz, 0) + 1

    with tile.TileContext(nc) as tc, ExitStack() as ctx:
        # per-block-size pools so tail blocks don't pay 512-wide slots
        xps = {sz: ctx.enter_context(tc.tile_pool(name=f"xp{sz}", bufs=DK * n))
               for sz, n in nsz.items()}
        h1ps = {sz: ctx.enter_context(tc.tile_pool(name=f"h1p{sz}",
                                                   bufs=n * (HC // P)))
                for sz, n in nsz.items()}
        w1p = ctx.enter_context(tc.tile_pool(name="w1p", bufs=HC // P))
        w2p = ctx.enter_context(tc.tile_pool(name="w2p", bufs=2))
        bp = ctx.enter_context(tc.tile_pool(name="bp", bufs=1))
        op = ctx.enter_context(tc.tile_pool(name="op", bufs=4))
        ps1 = ctx.enter_context(tc.tile_pool(name="ps1", bufs=3, space="PSUM"))
        ps2 = ctx.enter_context(tc.tile_pool(name="ps2", bufs=4, space="PSUM"))

        b1t = bp.tile([P, HM], f32)
        nc.sync.dma_start(out=b1t[:], in_=b1.rearrange("(m p) -> p m", p=P))
        b2t = bp.tile([P, DM], f32)
        nc.sync.dma_start(out=b2t[:], in_=b2.rearrange("(m p) -> p m", p=P))
        zbias = bp.tile([P, 1], f32)
        nc.vector.memset(zbias[:], 0.0)

        def load_w1_strip(gm, split=False):
            w1s = w1p.tile([P, D], f32r)
            if split:
                # per-k-tile DMAs spread across HW queues (startup latency)
                for k in range(DK):
                    nc.sync.dma_start(out=w1s[:, k * P:(k + 1) * P],
                                      in_=w1[k, :, gm * P:(gm + 1) * P])
            else:
                nc.sync.dma_start(
                    out=w1s[:].rearrange("p (k q) -> p k q", q=P),
                    in_=w1[:, :, gm * P:(gm + 1) * P].rearrange("k p q -> p k q"),
                )
            return w1s

        # DMA emission order sets scheduler priority: first weight strip,
        # then xT token blocks t-major, so the first matmul starts after
        # ~2.5 MB instead of after the whole 10 MB xT panel.
        w1s0 = load_w1_strip(0, split=True)
        xts = {}
        for t, (toff, tsz) in enumerate(blocks):
            for k in range(DK):
                xt = xps[tsz].tile([P, tsz], f32r)
                nc.sync.dma_start(out=xt[:],
                                  in_=xT[k * P:(k + 1) * P, toff:toff + tsz])
                xts[(k, t)] = xt
            if t == 0:
                w1strips0 = [w1s0] + [load_w1_strip(m, split=True) for m in range(1, HC // P)]

        def mm1(c, m, t, w1s, h1):
            gm = c * (HC // P) + m
            toff, tsz = blocks[t]
            ps = ps1.tile([P, tsz], f32)
            for k in range(DK):
                nc.tensor.matmul(ps[:],
                                 lhsT=w1s[:, k * P:(k + 1) * P],
                                 rhs=xts[(k, t)][:],
                                 start=(k == 0), stop=(k == DK - 1))
            ht = h1ps[tsz].tile([P, tsz], f32r)
            nc.scalar.activation(ht[:], ps[:],
                                 AF.Gelu_apprx_tanh,
                                 bias=b1t[:, gm:gm + 1])
            h1[(m, t)] = ht

        for c in range(NCH):
            h1 = {}
            # ---- mm1: h1[c] = gelu(w1[:, chunk].T @ xT + b1[chunk]) ----
            if c == 0:
                # t-outer: compute starts as soon as the first token block
                # lands (all 8 chunk-0 weight strips are resident).
                for t in range(nt):
                    for m in range(HC // P):
                        mm1(c, m, t, w1strips0[m], h1)
            else:
                for m in range(HC // P):
                    w1s = load_w1_strip(c * (HC // P) + m)
                    for t in range(nt):
                        mm1(c, m, t, w1s, h1)
            # ---- mm2 partial: yT += w2[chunk].T @ h1[c]  (+ b2 once) ----
            kt0 = c * (HC // P)
            for m2 in range(DM):
                w2s = w2p.tile([P, HC], f32r)
                nc.sync.dma_start(
                    out=w2s[:].rearrange("p (k q) -> p k q", q=P),
                    in_=w2[kt0:kt0 + HC // P, :, m2 * P:(m2 + 1) * P]
                        .rearrange("k p q -> p k q"),
                )
                for t, (toff, tsz) in enumerate(blocks):
                    ps = ps2.tile([P, tsz], f32)
                    for k2 in range(HC // P):
                        nc.tensor.matmul(ps[:],
                                         lhsT=w2s[:, k2 * P:(k2 + 1) * P],
                                         rhs=h1[(k2, t)][:],
                                         start=(k2 == 0), stop=(k2 == HC // P - 1))
                    ot = op.tile([P, TB], f32)
                    if c == 0:
                        # fold the (once-only) b2 add into the eviction
                        nc.scalar.activation(ot[:, :tsz], ps[:],
                                             AF.Identity,
                                             bias=b2t[:, m2:m2 + 1])
                    elif t % 2 == 0:
                        nc.vector.tensor_copy(ot[:, :tsz], ps[:])
                    else:
                        # alternate engines so the eviction chain keeps up
                        nc.scalar.activation(ot[:, :tsz], ps[:], AF.Identity,
                                             bias=zbias[:])
                    nc.gpsimd.dma_start(
                        out=yT[m2 * P:(m2 + 1) * P, toff:toff + tsz],
                        in_=ot[:, :tsz],
                        accum_op=(mybir.AluOpType.bypass if c == 0
                                  else mybir.AluOpType.add),
                    )
    nc.compile()
    return nc


def _get_program(C):
    if C not in _prog_cache:
        _prog_cache[C] = build_program(C)
    return _prog_cache[C]


def kernel(x, gate_w, w1, b1, w2, b2):
    global LAST_RESULTS
    x = np.asarray(x, dtype=np.float32)
    gate_w = np.asarray(gate_w, dtype=np.float32)
    w1 = np.asarray(w1, dtype=np.float32)
    b1 = np.asarray(b1, dtype=np.float32)
    w2 = np.asarray(w2, dtype=np.float32)
    b2 = np.asarray(b2, dtype=np.float32)

    T = B * L
    xf = x.reshape(T, D)

    # ---- router (host): softmax over experts, top-2, renormalize ----
    logits = xf @ gate_w                       # (T, E)
    mx = logits.max(axis=-1, keepdims=True)
    p = np.exp(logits - mx, dtype=np.float32)
    p /= p.sum(axis=-1, keepdims=True)
    idx = np.argsort(-p, axis=-1, kind="stable")[:, :K]       # top-2, ties->low idx
    wts = np.take_along_axis(p, idx, axis=-1)
    wts = wts / wts.sum(axis=-1, keepdims=True)

    # ---- dispatch: gather tokens per expert, pad to capacity ----
    tok_lists, coef_lists = [], []
    for e in range(E):
        mask = (idx == e)
        toks = np.nonzero(mask.any(axis=-1))[0]
        coefs = wts[mask.any(axis=-1)][mask[mask.any(axis=-1)]]  # per-token weight
        # simpler/safer: recompute coefs aligned with toks
        coefs = (wts * mask).sum(axis=-1)[toks].astype(np.float32)
        tok_lists.append(toks)
        coef_lists.append(coefs)
    cmax = max(len(t) for t in tok_lists)
    # SBUF residency (xT + h1 panels) caps a single run at ~2176 tokens per
    # expert; extremely imbalanced routing falls back to multiple runs.
    MAXC = int(os.environ.get("BASSK_MAXC", "2176"))
    ngroups = max(1, -(-cmax // MAXC))
    gmax = -(-cmax // ngroups)
    C = max(256, -(-gmax // 128) * 128)

    nc = _get_program(C)

    if TRACE:
        _install_prof_shim()
    from concourse.bass_utils import run_bass_kernel_spmd

    wmaps = []
    for e in range(E):
        wmaps.append({
            "w1": np.ascontiguousarray(w1[e]).reshape(DK, P, H),
            "w2": np.ascontiguousarray(w2[e]).reshape(HM, P, D),
            "b1": np.ascontiguousarray(b1[e]),
            "b2": np.ascontiguousarray(b2[e]),
        })

    out = np.zeros((T, D), dtype=np.float32)
    for g in range(ngroups):
        in_maps = []
        gtoks = []
        for e in range(E):
            toks = tok_lists[e][g * gmax:(g + 1) * gmax]
            gtoks.append(toks)
            xTe = np.zeros((D, C), dtype=np.float32)
            xTe[:, :len(toks)] = xf[toks].T
            in_maps.append({"xT": xTe, **wmaps[e]})

        res = run_bass_kernel_spmd(nc, in_maps, list(range(NCORES)),
                                   trace=TRACE)
        LAST_RESULTS = res

        # ---- combine (host): out[tok] += coef * y ----
        for e in range(E):
            toks = gtoks[e]
            if len(toks) == 0:
                continue
            ye = res.results[e]["yT"][:, :len(toks)].T       # (cnt, D)
            coefs = coef_lists[e][g * gmax:(g + 1) * gmax]
            out[toks] += coefs[:, None] * ye
    out = out.reshape(B, L, D)

    # ---- aux loss (host) ----
    f = np.zeros(E, dtype=np.float64)
    for e in range(E):
        f[e] = (idx == e).sum()
    f /= (T * K)
    Pm = p.mean(axis=0, dtype=np.float64)
    aux_loss = np.float32(E * np.sum(f * Pm))

    return out, aux_loss


# revision 20
# speedup vs baseline: 1.0708x; 1.0708x over previous
"""Trainium2 Bass kernel for an 8-expert top-2 MoE layer (expert-parallel).

Problem (hardcoded): x:(4,2048,1024) f32, gate_w:(1024,8), w1:(8,1024,4096),
b1:(8,4096), w2:(8,4096,1024), b2:(8,1024).  Returns (out, aux_loss) like the
reference:
    logits = x @ gate_w ; probs = softmax ; top-2 renormalized
    out    = sum_e coef_e * (gelu(x @ w1[e] + b1[e]) @ w2[e] + b2[e])
    aux    = E * sum(f * P)

Strategy:
  * Router / softmax / top-2 / aux-loss on host (0.1% of the FLOPs).
  * Expert-parallel: core e runs expert e's FFN over the tokens routed to it
    (gathered + padded to capacity C on host, "all-to-all dispatch" done
    host-side since the kernel receives full inputs anyway).
  * Per core: yT = w2.T @ gelu(w1.T @ xT + b1) + b2 with all matmuls in
    float32r (full-rate fp32 path of the PE, ~1e-4 rel err).
    Layouts are chosen so no transposes are ever needed on device:
    stationary = weight tiles as stored, moving = token panels.
  * H is processed in 4 chunks of 1024 so the h1 activations for one chunk
    (all token blocks) stay SBUF-resident; the second matmul accumulates
    chunk partials straight into DRAM via DMA accum_op=add.
  * Host combines: out[tok] += coef * yT.T, weighted by routing weights.
"""
import os
import sys

sys.path.insert(0, "/opt/trn_rl_repo")

from contextlib import ExitStack

import numpy as np

import concourse.bacc as bacc
import concourse.bass as bass
import concourse.tile as tile
from concourse import mybir

B, L, D = 4, 2048, 1024
E, H, K = 8, 4096, 2
P = 128
NCORES = 8
TB = 512            # token block (matmul moving free dim; one PSUM bank)
HC = 1024           # H chunk held in SBUF (8 k-tiles)
DK = D // P         # 8 k-tiles over D
HM = H // P         # 32 m-tiles over H
DM = D // P         # 8 m-tiles over D
NCH = H // HC       # 4 H-chunks

TRACE = bool(int(os.environ.get("BASSK_TRACE", "0")))
LAST_RESULTS = None

_prog_cache = {}


def _install_prof_shim():
    """Enable NTFF profiling under axon (the agent image's antenv lacks the
    hook) and keep profile artifacts local."""
    import types
    import concourse.bass_utils as bu

    if "antenv.axon_hooks" not in sys.modules:
        holder = {"hook": None}
        mod = types.ModuleType("antenv.axon_hooks")
        mod.set_axon_ntff_profile_hook = lambda h: holder.update(hook=h)
        mod.get_axon_ntff_profile_hook = lambda: holder["hook"]
        sys.modules["antenv.axon_hooks"] = mod
        from trn_agent_boot.trn_boot import _ntff_profile_via_ctypes

        mod.set_axon_ntff_profile_hook(
            _ntff_profile_via_ctypes("/opt/axon/libaxon_pjrt.so")
        )
        bu.upload_artifacts = lambda tmpdir: f"local://{tmpdir}"


def build_program(C):
    """One SPMD program: yT = w2.T @ gelu(w1.T @ xT + b1) + b2, xT:[D,C]."""
    # Decompose C into token blocks of 512/384/256 — fp32r matmuls need a
    # moving free dim >=256 to run at full rate (1 cycle/row).
    assert C % 128 == 0 and C >= 256
    sizes = [TB] * (C // TB)
    r = C % TB
    if r == 128:
        sizes = sizes[:-1] + [384, 256]
    elif r:
        sizes.append(r)
    blocks = []
    off = 0
    for sz in sizes:
        blocks.append((off, sz))
        off += sz
    assert off == C
    nt = len(blocks)
    f32r = mybir.dt.float32r
    f32 = mybir.dt.float32
    AF = mybir.ActivationFunctionType

    nc = bacc.Bacc("TRN2", target_bir_lowering=False, debug=False,
                   num_devices=NCORES)
    xT = nc.dram_tensor("xT", [D, C], f32r, kind="ExternalInput").ap()
    w1 = nc.dram_tensor("w1", [DK, P, H], f32r, kind="ExternalInput").ap()
    w2 = nc.dram_tensor("w2", [HM, P, D], f32r, kind="ExternalInput").ap()
    b1 = nc.dram_tensor("b1", [H], f32, kind="ExternalInput").ap()
    b2 = nc.dram_tensor("b2", [D], f32, kind="ExternalInput").ap()
    yT = nc.dram_tensor("yT", [D, C], f32, kind="ExternalOutput").ap()

    nsz = {}
    for _, sz in blocks:
        nsz[sz] = nsz.get(sz, 0) + 1

    with tile.TileContext(nc) as tc, ExitStack() as ctx:
        # per-block-size pools so tail blocks don't pay 512-wide slots
        xps = {sz: ctx.enter_context(tc.tile_pool(name=f"xp{sz}", bufs=DK * n))
               for sz, n in nsz.items()}
        h1ps = {sz: ctx.enter_context(tc.tile_pool(name=f"h1p{sz}",
                                                   bufs=n * (HC // P)))
                for sz, n in nsz.items()}
        w1p = ctx.enter_context(tc.tile_pool(name="w1p", bufs=HC // P))
        w2p = ctx.enter_context(tc.tile_pool(name="w2p", bufs=2))
        bp = ctx.enter_context(tc.tile_pool(name="bp", bufs=1))
        op = ctx.enter_context(tc.tile_pool(name="op", bufs=4))
        ps1 = ctx.enter_context(tc.tile_pool(name="ps1", bufs=3, space="PSUM"))
        ps2 = ctx.enter_context(tc.tile_pool(name="ps2", bufs=4, space="PSUM"))

        b1t = bp.tile([P, HM], f32)
        nc.sync.dma_start(out=b1t[:], in_=b1.rearrange("(m p) -> p m", p=P))
        b2t = bp.tile([P, DM], f32)
        nc.sync.dma_start(out=b2t[:], in_=b2.rearrange("(m p) -> p m", p=P))
        zbias = bp.tile([P, 1], f32)
        nc.vector.memset(zbias[:], 0.0)

        def load_w1_strip(gm, split=False):
            w1s = w1p.tile([P, D], f32r)
            if split:
                # per-k-tile DMAs spread across HW queues (startup latency)
                for k in range(DK):
                    nc.sync.dma_start(out=w1s[:, k * P:(k + 1) * P],
                                      in_=w1[k, :, gm * P:(gm + 1) * P])
            else:
                nc.sync.dma_start(
                    out=w1s[:].rearrange("p (k q) -> p k q", q=P),
                    in_=w1[:, :, gm * P:(gm + 1) * P].rearrange("k p q -> p k q"),
                )
            return w1s

        # DMA emission order sets scheduler priority: first weight strip,
        # then xT token blocks t-major, so the first matmul starts after
        # ~2.5 MB instead of after the whole 10 MB xT panel.
        w1s0 = load_w1_strip(0)
        xts = {}
        for t, (toff, tsz) in enumerate(blocks):
            for k in range(DK):
                xt = xps[tsz].tile([P, tsz], f32r)
                nc.sync.dma_start(out=xt[:],
                                  in_=xT[k * P:(k + 1) * P, toff:toff + tsz])
                xts[(k, t)] = xt
            if t == 0:
                w1strips0 = [w1s0] + [load_w1_strip(m) for m in range(1, HC // P)]

        def mm1(c, m, t, w1s, h1):
            gm = c * (HC // P) + m
            toff, tsz = blocks[t]
            ps = ps1.tile([P, tsz], f32)
            for k in range(DK):
                nc.tensor.matmul(ps[:],
                                 lhsT=w1s[:, k * P:(k + 1) * P],
                                 rhs=xts[(k, t)][:],
                                 start=(k == 0), stop=(k == DK - 1))
            ht = h1ps[tsz].tile([P, tsz], f32r)
            nc.scalar.activation(ht[:], ps[:],
                                 AF.Gelu_apprx_tanh,
                                 bias=b1t[:, gm:gm + 1])
            h1[(m, t)] = ht

        for c in range(NCH):
            h1 = {}
            # ---- mm1: h1[c] = gelu(w1[:, chunk].T @ xT + b1[chunk]) ----
            if c == 0:
                # t-outer: compute starts as soon as the first token block
                # lands (all 8 chunk-0 weight strips are resident).
                for t in range(nt):
                    for m in range(HC // P):
                        mm1(c, m, t, w1strips0[m], h1)
            else:
                for m in range(HC // P):
                    w1s = load_w1_strip(c * (HC // P) + m)
                    for t in range(nt):
                        mm1(c, m, t, w1s, h1)
            # ---- mm2 partial: yT += w2[chunk].T @ h1[c]  (+ b2 once) ----
            kt0 = c * (HC // P)
            for m2 in range(DM):
                w2s = w2p.tile([P, HC], f32r)
                nc.sync.dma_start(
                    out=w2s[:].rearrange("p (k q) -> p k q", q=P),
                    in_=w2[kt0:kt0 + HC // P, :, m2 * P:(m2 + 1) * P]
                        .rearrange("k p q -> p k q"),
                )
                for t, (toff, tsz) in enumerate(blocks):
                    ps = ps2.tile([P, tsz], f32)
                    for k2 in range(HC // P):
                        nc.tensor.matmul(ps[:],
                                         lhsT=w2s[:, k2 * P:(k2 + 1) * P],
                                         rhs=h1[(k2, t)][:],
                                         start=(k2 == 0), stop=(k2 == HC // P - 1))
                    ot = op.tile([P, TB], f32)
                    if c == 0:
                        # fold the (once-only) b2 add into the eviction
                        nc.scalar.activation(ot[:, :tsz], ps[:],
                                             AF.Identity,
                                             bias=b2t[:, m2:m2 + 1])
                    elif t % 2 == 0:
                        nc.vector.tensor_copy(ot[:, :tsz], ps[:])
                    else:
                        # alternate engines so the eviction chain keeps up
                        nc.scalar.activation(ot[:, :tsz], ps[:], AF.Identity,
                                             bias=zbias[:])
                    nc.gpsimd.dma_start(
                        out=yT[m2 * P:(m2 + 1) * P, toff:toff + tsz],
                        in_=ot[:, :tsz],
                        accum_op=(mybir.AluOpType.bypass if c == 0
                                  else mybir.AluOpType.add),
                    )
    nc.compile()
    return nc


def _get_program(C):
    if C not in _prog_cache:
        _prog_cache[C] = build_program(C)
    return _prog_cache[C]


def kernel(x, gate_w, w1, b1, w2, b2):
    global LAST_RESULTS
    x = np.asarray(x, dtype=np.float32)
    gate_w = np.asarray(gate_w, dtype=np.float32)
    w1 = np.asarray(w1, dtype=np.float32)
    b1 = np.asarray(b1, dtype=np.float32)
    w2 = np.asarray(w2, dtype=np.float32)
    b2 = np.asarray(b2, dtype=np.float32)

    T = B * L
    xf = x.reshape(T, D)

    # ---- router (host): softmax over experts, top-2, renormalize ----
    logits = xf @ gate_w                       # (T, E)
    mx = logits.max(axis=-1, keepdims=True)
    p = np.exp(logits - mx, dtype=np.float32)
    p /= p.sum(axis=-1, keepdims=True)
    idx = np.argsort(-p, axis=-1, kind="stable")[:, :K]       # top-2, ties->low idx
    wts = np.take_along_axis(p, idx, axis=-1)
    wts = wts / wts.sum(axis=-1, keepdims=True)

    # ---- dispatch: gather tokens per expert, pad to capacity ----
    tok_lists, coef_lists = [], []
    for e in range(E):
        mask = (idx == e)
        toks = np.nonzero(mask.any(axis=-1))[0]
        coefs = wts[mask.any(axis=-1)][mask[mask.any(axis=-1)]]  # per-token weight
        # simpler/safer: recompute coefs aligned with toks
        coefs = (wts * mask).sum(axis=-1)[toks].astype(np.float32)
        tok_lists.append(toks)
        coef_lists.append(coefs)
    cmax = max(len(t) for t in tok_lists)
    # SBUF residency (xT + h1 panels) caps a single run at ~2176 tokens per
    # expert; extremely imbalanced routing falls back to multiple runs.
    MAXC = int(os.environ.get("BASSK_MAXC", "2176"))
    ngroups = max(1, -(-cmax // MAXC))
    gmax = -(-cmax // ngroups)
    C = max(256, -(-gmax // 128) * 128)

    nc = _get_program(C)

    if TRACE:
        _install_prof_shim()
    from concourse.bass_utils import run_bass_kernel_spmd

    wmaps = []
    for e in range(E):
        wmaps.append({
            "w1": np.ascontiguousarray(w1[e]).reshape(DK, P, H),
            "w2": np.ascontiguousarray(w2[e]).reshape(HM, P, D),
            "b1": np.ascontiguousarray(b1[e]),
            "b2": np.ascontiguousarray(b2[e]),
        })

    out = np.zeros((T, D), dtype=np.float32)
    for g in range(ngroups):
        in_maps = []
        gtoks = []
        for e in range(E):
            toks = tok_lists[e][g * gmax:(g + 1) * gmax]
            gtoks.append(toks)
            xTe = np.zeros((D, C), dtype=np.float32)
            xTe[:, :len(toks)] = xf[toks].T
            in_maps.append({"xT": xTe, **wmaps[e]})

        res = run_bass_kernel_spmd(nc, in_maps, list(range(NCORES)),
                                   trace=TRACE)
        LAST_RESULTS = res

        # ---- combine (host): out[tok] += coef * y ----
        for e in range(E):
            toks = gtoks[e]
            if len(toks) == 0:
                continue
            ye = res.results[e]["yT"][:, :len(toks)].T       # (cnt, D)
            coefs = coef_lists[e][g * gmax:(g + 1) * gmax]
            out[toks] += coefs[:, None] * ye
    out = out.reshape(B, L, D)

    # ---- aux loss (host) ----
    f = np.zeros(E, dtype=np.float64)
    for e in range(E):
        f[e] = (idx == e).sum()
    f /= (T * K)
    Pm = p.mean(axis=0, dtype=np.float64)
    aux_loss = np.float32(E * np.sum(f * Pm))

    return out, aux_loss


# revision 23
# speedup vs baseline: 1.1290x; 1.0543x over previous
"""Trainium2 Bass kernel for an 8-expert top-2 MoE layer (expert-parallel).

Problem (hardcoded): x:(4,2048,1024) f32, gate_w:(1024,8), w1:(8,1024,4096),
b1:(8,4096), w2:(8,4096,1024), b2:(8,1024).  Returns (out, aux_loss) like the
reference:
    logits = x @ gate_w ; probs = softmax ; top-2 renormalized
    out    = sum_e coef_e * (gelu(x @ w1[e] + b1[e]) @ w2[e] + b2[e])
    aux    = E * sum(f * P)

Strategy:
  * Router / softmax / top-2 / aux-loss on host (0.1% of the FLOPs).
  * Expert-parallel: core e runs expert e's FFN over the tokens routed to it
    (gathered + padded to capacity C on host, "all-to-all dispatch" done
    host-side since the kernel receives full inputs anyway).
  * Per core: yT = w2.T @ gelu(w1.T @ xT + b1) + b2 with all matmuls in
    float32r (full-rate fp32 path of the PE, ~1e-4 rel err).
    Layouts are chosen so no transposes are ever needed on device:
    stationary = weight tiles as stored, moving = token panels.
  * H is processed in 4 chunks of 1024 so the h1 activations for one chunk
    (all token blocks) stay SBUF-resident; the second matmul accumulates
    chunk partials straight into DRAM via DMA accum_op=add.
  * Host combines: out[tok] += coef * yT.T, weighted by routing weights.
"""
import os
import sys

sys.path.insert(0, "/opt/trn_rl_repo")

from contextlib import ExitStack

import numpy as np

import concourse.bacc as bacc
import concourse.bass as bass
import concourse.tile as tile
from concourse import mybir

B, L, D = 4, 2048, 1024
E, H, K = 8, 4096, 2
P = 128
NCORES = 8
TB = 512            # token block (matmul moving free dim; one PSUM bank)
HC = 1024           # H chunk held in SBUF (8 k-tiles)
DK = D // P         # 8 k-tiles over D
HM = H // P         # 32 m-tiles over H
DM = D // P         # 8 m-tiles over D
NCH = H // HC       # 4 H-chunks

TRACE = bool(int(os.environ.get("BASSK_TRACE", "0")))
LAST_RESULTS = None

_prog_cache = {}


def _install_prof_shim():
    """Enable NTFF profiling under axon (the agent image's antenv lacks the
    hook) and keep profile artifacts local."""
    import types
    import concourse.bass_utils as bu

    if "antenv.axon_hooks" not in sys.modules:
        holder = {"hook": None}
        mod = types.ModuleType("antenv.axon_hooks")
        mod.set_axon_ntff_profile_hook = lambda h: holder.update(hook=h)
        mod.get_axon_ntff_profile_hook = lambda: holder["hook"]
        sys.modules["antenv.axon_hooks"] = mod
        from trn_agent_boot.trn_boot import _ntff_profile_via_ctypes

        mod.set_axon_ntff_profile_hook(
            _ntff_profile_via_ctypes("/opt/axon/libaxon_pjrt.so")
        )
        bu.upload_artifacts = lambda tmpdir: f"local://{tmpdir}"


def build_program(C):
    """One SPMD program: yT = w2.T @ gelu(w1.T @ xT + b1) + b2, xT:[D,C]."""
    # Decompose C into token blocks, each 256..512 wide — fp32r matmuls need
    # a moving free dim >=256 to run at full rate (1 cycle/row).
    assert C >= 256 and C % 8 == 0
    sizes = []
    rem = C
    while rem > 768:
        sizes.append(TB)
        rem -= TB
    if rem > TB:
        sizes += [rem - 256, 256]
    else:
        sizes.append(rem)
    blocks = []
    off = 0
    for sz in sizes:
        blocks.append((off, sz))
        off += sz
    assert off == C and all(256 <= s <= TB and s % 8 == 0 for s in sizes)
    nt = len(blocks)
    f32r = mybir.dt.float32r
    f32 = mybir.dt.float32
    AF = mybir.ActivationFunctionType

    nc = bacc.Bacc("TRN2", target_bir_lowering=False, debug=False,
                   num_devices=NCORES)
    xT = nc.dram_tensor("xT", [D, C], f32r, kind="ExternalInput").ap()
    w1 = nc.dram_tensor("w1", [DK, P, H], f32r, kind="ExternalInput").ap()
    w2 = nc.dram_tensor("w2", [HM, P, D], f32r, kind="ExternalInput").ap()
    b1 = nc.dram_tensor("b1", [H], f32, kind="ExternalInput").ap()
    b2 = nc.dram_tensor("b2", [D], f32, kind="ExternalInput").ap()
    yT = nc.dram_tensor("yT", [D, C], f32, kind="ExternalOutput").ap()

    nsz = {}
    for _, sz in blocks:
        nsz[sz] = nsz.get(sz, 0) + 1

    with tile.TileContext(nc) as tc, ExitStack() as ctx:
        # per-block-size pools so tail blocks don't pay 512-wide slots
        xps = {sz: ctx.enter_context(tc.tile_pool(name=f"xp{sz}", bufs=DK * n))
               for sz, n in nsz.items()}
        h1ps = {sz: ctx.enter_context(tc.tile_pool(name=f"h1p{sz}",
                                                   bufs=n * (HC // P)))
                for sz, n in nsz.items()}
        w1p = ctx.enter_context(tc.tile_pool(name="w1p", bufs=HC // P))
        w2p = ctx.enter_context(tc.tile_pool(name="w2p", bufs=3))
        bp = ctx.enter_context(tc.tile_pool(name="bp", bufs=1))
        op = ctx.enter_context(tc.tile_pool(name="op", bufs=6))
        ps1 = ctx.enter_context(tc.tile_pool(name="ps1", bufs=3, space="PSUM"))
        ps2 = ctx.enter_context(tc.tile_pool(name="ps2", bufs=4, space="PSUM"))

        b1t = bp.tile([P, HM], f32)
        nc.sync.dma_start(out=b1t[:], in_=b1.rearrange("(m p) -> p m", p=P))
        b2t = bp.tile([P, DM], f32)
        nc.sync.dma_start(out=b2t[:], in_=b2.rearrange("(m p) -> p m", p=P))
        zbias = bp.tile([P, 1], f32)
        nc.vector.memset(zbias[:], 0.0)

        def load_w1_strip(gm, split=False):
            w1s = w1p.tile([P, D], f32r)
            if split:
                # per-k-tile DMAs spread across HW queues (startup latency)
                for k in range(DK):
                    nc.sync.dma_start(out=w1s[:, k * P:(k + 1) * P],
                                      in_=w1[k, :, gm * P:(gm + 1) * P])
            else:
                nc.sync.dma_start(
                    out=w1s[:].rearrange("p (k q) -> p k q", q=P),
                    in_=w1[:, :, gm * P:(gm + 1) * P].rearrange("k p q -> p k q"),
                )
            return w1s

        # DMA emission order sets scheduler priority: first weight strip,
        # then xT token blocks t-major, so the first matmul starts after
        # ~2.5 MB instead of after the whole 10 MB xT panel.
        w1s0 = load_w1_strip(0)
        xts = {}
        for t, (toff, tsz) in enumerate(blocks):
            for k in range(DK):
                xt = xps[tsz].tile([P, tsz], f32r)
                nc.sync.dma_start(out=xt[:],
                                  in_=xT[k * P:(k + 1) * P, toff:toff + tsz])
                xts[(k, t)] = xt
            if t == 0:
                w1strips0 = [w1s0] + [load_w1_strip(m) for m in range(1, HC // P)]

        def mm1(c, m, t, w1s, h1):
            gm = c * (HC // P) + m
            toff, tsz = blocks[t]
            ps = ps1.tile([P, tsz], f32)
            for k in range(DK):
                nc.tensor.matmul(ps[:],
                                 lhsT=w1s[:, k * P:(k + 1) * P],
                                 rhs=xts[(k, t)][:],
                                 start=(k == 0), stop=(k == DK - 1))
            ht = h1ps[tsz].tile([P, tsz], f32r)
            nc.scalar.activation(ht[:], ps[:],
                                 AF.Gelu_apprx_tanh,
                                 bias=b1t[:, gm:gm + 1])
            h1[(m, t)] = ht

        for c in range(NCH):
            h1 = {}
            # ---- mm1: h1[c] = gelu(w1[:, chunk].T @ xT + b1[chunk]) ----
            if c == 0:
                # t-outer: compute starts as soon as the first token block
                # lands (all 8 chunk-0 weight strips are resident).
                for t in range(nt):
                    for m in range(HC // P):
                        mm1(c, m, t, w1strips0[m], h1)
            else:
                for m in range(HC // P):
                    w1s = load_w1_strip(c * (HC // P) + m)
                    for t in range(nt):
                        mm1(c, m, t, w1s, h1)
            # ---- mm2 partial: yT += w2[chunk].T @ h1[c]  (+ b2 once) ----
            kt0 = c * (HC // P)
            for m2 in range(DM):
                w2s = w2p.tile([P, HC], f32r)
                nc.sync.dma_start(
                    out=w2s[:].rearrange("p (k q) -> p k q", q=P),
                    in_=w2[kt0:kt0 + HC // P, :, m2 * P:(m2 + 1) * P]
                        .rearrange("k p q -> p k q"),
                )
                for t, (toff, tsz) in enumerate(blocks):
                    ps = ps2.tile([P, tsz], f32)
                    for k2 in range(HC // P):
                        nc.tensor.matmul(ps[:],
                                         lhsT=w2s[:, k2 * P:(k2 + 1) * P],
                                         rhs=h1[(k2, t)][:],
                                         start=(k2 == 0), stop=(k2 == HC // P - 1))
                    ot = op.tile([P, TB], f32)
                    if c == 0:
                        # fold the (once-only) b2 add into the eviction
                        nc.scalar.activation(ot[:, :tsz], ps[:],
                                             AF.Identity,
                                             bias=b2t[:, m2:m2 + 1])
                    elif t % 2 == 0:
                        nc.vector.tensor_copy(ot[:, :tsz], ps[:])
                    else:
                        # alternate engines so the eviction chain keeps up
                        nc.scalar.activation(ot[:, :tsz], ps[:], AF.Identity,
                                             bias=zbias[:])
                    nc.gpsimd.dma_start(
                        out=yT[m2 * P:(m2 + 1) * P, toff:toff + tsz],
                        in_=ot[:, :tsz],
                        accum_op=(mybir.AluOpType.bypass if c == 0
                                  else mybir.AluOpType.add),
                    )
    nc.compile()
    return nc


def _get_program(C):
    if C not in _prog_cache:
        _prog_cache[C] = build_program(C)
    return _prog_cache[C]


def kernel(x, gate_w, w1, b1, w2, b2):
    global LAST_RESULTS
    x = np.asarray(x, dtype=np.float32)
    gate_w = np.asarray(gate_w, dtype=np.float32)
    w1 = np.asarray(w1, dtype=np.float32)
    b1 = np.asarray(b1, dtype=np.float32)
    w2 = np.asarray(w2, dtype=np.float32)
    b2 = np.asarray(b2, dtype=np.float32)

    T = B * L
    xf = x.reshape(T, D)

    # ---- router (host): softmax over experts, top-2, renormalize ----
    logits = xf @ gate_w                       # (T, E)
    mx = logits.max(axis=-1, keepdims=True)
    p = np.exp(logits - mx, dtype=np.float32)
    p /= p.sum(axis=-1, keepdims=True)
    idx = np.argsort(-p, axis=-1, kind="stable")[:, :K]       # top-2, ties->low idx
    wts = np.take_along_axis(p, idx, axis=-1)
    wts = wts / wts.sum(axis=-1, keepdims=True)

    # ---- dispatch: gather tokens per expert, pad to capacity ----
    tok_lists, coef_lists = [], []
    for e in range(E):
        mask = (idx == e)
        toks = np.nonzero(mask.any(axis=-1))[0]
        coefs = wts[mask.any(axis=-1)][mask[mask.any(axis=-1)]]  # per-token weight
        # simpler/safer: recompute coefs aligned with toks
        coefs = (wts * mask).sum(axis=-1)[toks].astype(np.float32)
        tok_lists.append(toks)
        coef_lists.append(coefs)
    cmax = max(len(t) for t in tok_lists)
    # SBUF residency (xT + h1 panels) caps a single run at ~2176 tokens per
    # expert; extremely imbalanced routing falls back to multiple runs.
    MAXC = int(os.environ.get("BASSK_MAXC", "2176"))
    ngroups = max(1, -(-cmax // MAXC))
    gmax = -(-cmax // ngroups)
    C = max(256, -(-gmax // 8) * 8)

    nc = _get_program(C)

    if TRACE:
        _install_prof_shim()
    from concourse.bass_utils import run_bass_kernel_spmd

    wmaps = []
    for e in range(E):
        wmaps.append({
            "w1": np.ascontiguousarray(w1[e]).reshape(DK, P, H),
            "w2": np.ascontiguousarray(w2[e]).reshape(HM, P, D),
            "b1": np.ascontiguousarray(b1[e]),
            "b2": np.ascontiguousarray(b2[e]),
        })

    out = np.zeros((T, D), dtype=np.float32)
    for g in range(ngroups):
        in_maps = []
        gtoks = []
        for e in range(E):
            toks = tok_lists[e][g * gmax:(g + 1) * gmax]
            gtoks.append(toks)
            xTe = np.zeros((D, C), dtype=np.float32)
            xTe[:, :len(toks)] = xf[toks].T
            in_maps.append({"xT": xTe, **wmaps[e]})

        res = run_bass_kernel_spmd(nc, in_maps, list(range(NCORES)),
                                   trace=TRACE)
        LAST_RESULTS = res

        # ---- combine (host): out[tok] += coef * y ----
        for e in range(E):
            toks = gtoks[e]
            if len(toks) == 0:
                continue
            ye = res.results[e]["yT"][:, :len(toks)].T       # (cnt, D)
            coefs = coef_lists[e][g * gmax:(g + 1) * gmax]
            out[toks] += coefs[:, None] * ye
    out = out.reshape(B, L, D)

    # ---- aux loss (host) ----
    f = np.zeros(E, dtype=np.float64)
    for e in range(E):
        f[e] = (idx == e).sum()
    f /= (T * K)
    Pm = p.mean(axis=0, dtype=np.float64)
    aux_loss = np.float32(E * np.sum(f * Pm))

    return out, aux_loss
